# revision 1
# baseline (speedup 1.0000x reference)
"""Trainium2 Bass kernel for nn_Cross_attention2 (dense transformer cross-attention).

Math (per batch b, head h), faithful to the reference module:
    Q = q @ W_h + b_h ; K = k @ W_h + b_h ; V = v @ W_h + b_h
    alpha = (Q K^T)/sqrt(512); masked -> -1e9; alpha /= sqrt(512); P = softmax(alpha)
    out[b, :, h*512:(h+1)*512] = P @ V

Device algorithm (algebraically identical post-softmax):
    What = [W_h; b_h]  (513 x 512, host-marshalled)
    Ghat = What @ What^T            (on device, rows 0..512 x cols 0..511 used)
    Z    = Ghat[:512, :513] @ [q^T; 1]  = G q^T + (W b) 1^T          [512, Lq]
    s^T  = k @ Z = (QK^T)^T minus per-q/constant shifts (softmax-invariant)
    P^T  = exp(s^T/512 + maskbias^T/512) ; maskbias = (1-mask)*(-1e9*sqrt(512))
    sums = column sums of P^T (per q);  V = v @ W_h + 1 b_h^T
    O    = (P^T)^T @ V, scaled per-partition by 1/sums
Sharding: batch data-parallel, 2 batches per core, all 8 heads per core.
All matmuls run as float32r (full PE rate, ~1e-4 matmul rel-err).
"""

import os
import sys
from contextlib import ExitStack

import numpy as np

for _p in ("/opt/trn_rl_repo",):
    if os.path.isdir(_p) and _p not in sys.path:
        sys.path.append(_p)

import concourse.bacc as bacc
import concourse.mybir as mybir
import concourse.tile as tile
from concourse.bass import ts
from concourse.bass_utils import run_bass_kernel_spmd

dt = mybir.dt

B, L, D, H = 16, 512, 512, 8
NCORES = 8
BGROUPS, HGROUPS = 2, 4          # core grid: 2 batch-groups x 4 head-groups
BPC = B // BGROUPS               # 8 batches per core
HPC = H // HGROUPS               # 2 heads per core
C = D // 128  # 128-row chunks per 512
NEG_MASK = -1e9 * float(np.sqrt(512.0))  # additive bias for masked entries (pre /512)

_CACHE = {}


def _build():
    nc = bacc.Bacc("TRN2", target_bir_lowering=False, debug=False, num_devices=NCORES)
    f32 = dt.float32
    f32r = dt.float32r
    f16 = dt.float16

    qT_d = nc.dram_tensor("qT", [BPC, D, L], f16, kind="ExternalInput").ap()
    kT_d = nc.dram_tensor("kT", [BPC, D, L], f16, kind="ExternalInput").ap()
    vT_d = nc.dram_tensor("vT", [BPC, D, L], f16, kind="ExternalInput").ap()
    mbT_d = nc.dram_tensor("mbT", [BPC, L, L], f32, kind="ExternalInput").ap()
    Wn_d = nc.dram_tensor("Wn", [HPC, D, D], f16, kind="ExternalInput").ap()
    WaT_d = nc.dram_tensor("WaT", [HPC, D, D + 1], f16, kind="ExternalInput").ap()
    ones_d = nc.dram_tensor("ones", [128, L], f16, kind="ExternalInput").ap()
    bb_d = nc.dram_tensor("bb", [HPC, 128, D], f32, kind="ExternalInput").ap()
    out_d = nc.dram_tensor("out", [BPC, L, HPC * D], f32, kind="ExternalOutput").ap()

    EXP = mybir.ActivationFunctionType.Exp

    with tile.TileContext(nc) as tc, ExitStack() as ctx:
        const = ctx.enter_context(tc.tile_pool(name="const", bufs=1))
        acts = ctx.enter_context(tc.tile_pool(name="acts", bufs=2))
        headp = ctx.enter_context(tc.tile_pool(name="headp", bufs=1))
        work = ctx.enter_context(tc.tile_pool(name="work", bufs=2))
        psb = ctx.enter_context(tc.tile_pool(name="psb", bufs=4, space="PSUM"))
        pso = ctx.enter_context(tc.tile_pool(name="pso", bufs=2, space="PSUM"))
        pss = ctx.enter_context(tc.tile_pool(name="pss", bufs=1, space="PSUM"))

        # weights first (PE can start on Ghat while activations stream in)
        WaTs = []
        for h in range(HPC):
            WaTs.append(headp.tile([128, C, D + 1], f16, tag=f"WaT{h}", name=f"WaTs{h}"))

        def load_WaT(h, strips=1):
            for c in range(C):
                n = strips if c == 0 else 1
                w = (D + 1) // n
                for j in range(n):
                    lo = j * w
                    hi = (D + 1) if j == n - 1 else (j + 1) * w
                    nc.sync.dma_start(
                        WaTs[h][:, c, lo:hi],
                        WaT_d[h, c * 128 : (c + 1) * 128, lo:hi],
                    )

        def load_q(b):
            tq = acts.tile([128, C, L], f16, tag="q", name=f"qTs{b}")
            nc.sync.dma_start(tq[:], qT_d[b].rearrange("(c p) q -> p c q", p=128))
            return tq

        def load_kvm(b, tq):
            tk = acts.tile([128, C, L], f16, tag="k", name=f"kTs{b}")
            nc.sync.dma_start(tk[:], kT_d[b].rearrange("(c p) q -> p c q", p=128))
            tv = acts.tile([128, C, L], f16, tag="v", name=f"vTs{b}")
            nc.sync.dma_start(tv[:], vT_d[b].rearrange("(c p) q -> p c q", p=128))
            tm = acts.tile([128, C, L], f32, tag="m", name=f"mbs{b}")
            nc.sync.dma_start(tm[:], mbT_d[b].rearrange("(c p) q -> p c q", p=128))
            return (tq, tk, tv, tm)

        def load_acts(b, slot):
            return load_kvm(b, load_q(b))

        load_WaT(0, strips=4)
        onesT = const.tile([128, L], f16, tag="ones")
        nc.sync.dma_start(onesT[:], ones_d[:])
        _tq0 = load_q(0)
        load_WaT(1)
        cur_acts = load_kvm(0, _tq0)
        Wns, bbs_l = [], []
        for h in range(HPC):
            w = headp.tile([128, C, D], f16, tag=f"Wn{h}", name=f"Wns{h}")
            nc.sync.dma_start(w[:], Wn_d[h].rearrange("(c p) d -> p c d", p=128))
            Wns.append(w)
            bb = headp.tile([128, D], f32, tag=f"bb{h}", name=f"bbs{h}")
            nc.sync.dma_start(bb[:], bb_d[h])
            bbs_l.append(bb)

        # Ghat_h = What_h @ What_h^T, resident for the whole kernel.
        # Chunk-outer order: the first matmuls need only 1/4 of What,
        # so PE starts as soon as the first weight chunk lands.
        Ghats, WbCols = [], []
        for h in range(HPC):
            Ghat = headp.tile([128, C, D], f16, tag=f"Ghat{h}", name=f"Ghat{h}")
            gpss = [psb.tile([128, D], f32, tag="big", name=f"gps{h}{t}") for t in range(C)]
            rps = pss.tile([1, D], f32, tag="row")
            for c in range(C):
                for t in range(C):
                    nc.tensor.matmul(
                        gpss[t][:], WaTs[h][:, c, ts(t, 128)], WaTs[h][:, c, 0:D],
                        start=(c == 0), stop=(c == C - 1),
                    )
                nc.tensor.matmul(
                    rps[:], WaTs[h][:, c, D : D + 1], WaTs[h][:, c, 0:D],
                    start=(c == 0), stop=(c == C - 1),
                )
            for t in range(C):
                nc.vector.tensor_copy(Ghat[:, t, :], gpss[t][:])
            Ghrow = headp.tile([1, D], f16, tag=f"Ghrow{h}", name=f"Ghrow{h}")
            nc.vector.tensor_copy(Ghrow[:], rps[:])
            # Wb as a per-partition column [128, C, 2]: bias operand for the
            # Z psum->sbuf copy (replaces a K=1 rank-1 matmul per Z tile)
            wbps = pss.tile([128, 8], f32, tag="sums", name=f"wbps{h}")
            for t in range(C):
                nc.tensor.matmul(
                    wbps[:, 2 * t : 2 * t + 2],
                    Ghrow[0:1, ts(t, 128)], onesT[0:1, 0:2],
                    start=(t == 0), stop=(t == C - 1),
                )
            WbCol = headp.tile([128, C, 2], f32, tag=f"WbCol{h}", name=f"WbCol{h}")
            for t in range(C):
                nc.vector.tensor_copy(WbCol[:, t, :], wbps[:, 2 * t : 2 * t + 2])
            Ghats.append(Ghat)
            WbCols.append(WbCol)

        def emit_sums_O(st, final=False):
            """sums + O for a finished iteration; tiny sums MMs interleaved 1:1
            with big O MMs so the PE activity monitor never sees a lull.
            Output bias b is folded in post-softmax (softmax rows sum to 1, so
            normalized P @ (1 b^T) = 1 b^T) via a DVE add."""
            PTsb, Vsb, b, h = st

            def out_tile(u, ops, rsb, ru, split=1):
                Osb = work.tile([128, D], f32, tag="O", bufs=3, name=f"Osb{u}")
                nc.scalar.mul(Osb[:], ops[:], rsb[:, 2 * ru : 2 * ru + 1])
                nc.vector.tensor_add(Osb[:], Osb[:], bbs_l[h][:])
                w = D // split
                for j in range(split):
                    nc.sync.dma_start(
                        out_d[b, ts(u, 128), h * D + j * w : h * D + (j + 1) * w],
                        Osb[:, j * w : (j + 1) * w],
                    )

            if not final:
                sums = pss.tile([128, 8], f32, tag="sums")
                rsb = work.tile([128, 8], f32, tag="rsb")
                order = []
                for u0 in range(0, C, 2):
                    for t in range(C):
                        order += [(u0, t), (u0 + 1, t)]
                opss = {}
                n = 0
                for u, t in order:
                    if t == 0:
                        opss[u] = pso.tile([128, D], f32, tag="o", name=f"ops{u}")
                    nc.tensor.matmul(
                        opss[u][:], PTsb[:, t, ts(u, 128)], Vsb[:, t, :],
                        start=(t == 0), stop=(t == C - 1),
                    )
                    nc.tensor.matmul(
                        sums[:, 2 * u : 2 * u + 2],
                        PTsb[:, t, ts(u, 128)], onesT[:, 0:2],
                        start=(n == 0), stop=(n == 2 * C * C - 1),
                    )
                    n += 2
                    if t == C - 1 and u + 1 == C:
                        nc.vector.reciprocal(rsb[:], sums[:])
                for u in range(C):
                    out_tile(u, opss[u], rsb, u)
                return

            # final flush: per-pair sums groups so the first outputs drain
            # while the second pair's matmuls still run on the PE
            for u0 in range(0, C, 2):
                sums = pss.tile([128, 8], f32, tag="sums", name=f"fsums{u0}")
                rsb = work.tile([128, 8], f32, tag="rsb", name=f"frsb{u0}")
                opss = {}
                n = 0
                for t in range(C):
                    for u in (u0, u0 + 1):
                        if t == 0:
                            opss[u] = pso.tile([128, D], f32, tag="o", name=f"fops{u}")
                        nc.tensor.matmul(
                            opss[u][:], PTsb[:, t, ts(u, 128)], Vsb[:, t, :],
                            start=(t == 0), stop=(t == C - 1),
                        )
                        nc.tensor.matmul(
                            sums[:, 2 * (u - u0) : 2 * (u - u0) + 2],
                            PTsb[:, t, ts(u, 128)], onesT[:, 0:2],
                            start=(n == 0), stop=(n == 2 * C - 1),
                        )
                        n += 1
                nc.vector.reciprocal(rsb[:], sums[:])
                for u in (u0, u0 + 1):
                    out_tile(u, opss[u], rsb, u - u0, split=2)

        pending = None
        for b in range(BPC):
            qTb, kTb, vTb, mbb = cur_acts
            if b + 1 < BPC:
                nxt_acts = load_acts(b + 1, (b + 1) % 2)
            for h in range(HPC):
                Ghat, WbCol = Ghats[h], WbCols[h]

                # Z = G q^T; the +Wb bias is folded into the psum->sbuf copy
                # as a per-partition ACT bias (free: the copy is needed anyway)
                Zsb = work.tile([128, C, L], f16, tag="Z")
                for t in range(C):
                    zps = psb.tile([128, L], f32, tag="big")
                    for c in range(C):
                        nc.tensor.matmul(
                            zps[:], Ghat[:, c, ts(t, 128)], qTb[:, c, :],
                            start=(c == 0), stop=(c == C - 1),
                        )
                    nc.scalar.activation(
                        Zsb[:, t, :], zps[:],
                        mybir.ActivationFunctionType.Identity,
                        bias=WbCol[:, t, 0:1], scale=1.0,
                    )

                # V = vT^T @ W (bias folded in post-softmax)
                Vsb = work.tile([128, C, D], f16, tag="V")
                for t in range(C):
                    vps = psb.tile([128, D], f32, tag="big")
                    for c in range(C):
                        nc.tensor.matmul(
                            vps[:], vTb[:, c, ts(t, 128)], Wns[h][:, c, :],
                            start=(c == 0), stop=(c == C - 1),
                        )
                    nc.vector.tensor_copy(Vsb[:, t, :], vps[:])

                # s^T = kT^T @ Z ; P^T = exp(s^T/512 + mb^T/512)
                PTsb = work.tile([128, C, L], f16, tag="PT")
                for t in range(C):
                    sps = psb.tile([128, L], f32, tag="big")
                    for c in range(C):
                        nc.tensor.matmul(
                            sps[:], kTb[:, c, ts(t, 128)], Zsb[:, c, :],
                            start=(c == 0), stop=(c == C - 1),
                        )
                    sm = work.tile([128, L], f32, tag="sm")
                    nc.vector.tensor_add(sm[:], sps[:], mbb[:, t, :])
                    nc.scalar.activation(PTsb[:, t, :], sm[:], EXP, scale=1.0 / 512.0)

                # software pipeline: sums/O of the previous iteration lands
                # here, after this iteration's PE work covered its exp latency
                if pending is not None:
                    emit_sums_O(pending)
                pending = (PTsb, Vsb, b, h)
            if b + 1 < BPC:
                cur_acts = nxt_acts

        emit_sums_O(pending, final=True)

    nc.compile()
    return nc


def _prep_inputs(query, key, value, mask, Wq, bq):
    f = np.float32
    h16 = np.float16
    qT = np.ascontiguousarray(np.asarray(query, f).transpose(0, 2, 1).astype(h16))
    kT = np.ascontiguousarray(np.asarray(key, f).transpose(0, 2, 1).astype(h16))
    vT = np.ascontiguousarray(np.asarray(value, f).transpose(0, 2, 1).astype(h16))
    mb = (1.0 - np.asarray(mask, f)) * f(NEG_MASK)  # [B, Lq, Lk]
    mbT = np.ascontiguousarray(mb.transpose(0, 2, 1))  # [B, Lk, Lq]
    Wn32 = np.asarray(Wq, f)
    Wn = np.ascontiguousarray(Wn32.astype(h16))
    What = np.concatenate([Wn32, np.asarray(bq, f)[:, None, :]], axis=1)  # [H, 513, 512]
    WaT = np.ascontiguousarray(What.transpose(0, 2, 1).astype(h16))  # [H, 512, 513]
    ones = np.ones((128, L), h16)
    bb = np.broadcast_to(np.asarray(bq, f)[:, None, :], (H, 128, D)).copy()

    in_maps = []
    for c in range(NCORES):
        gb, gh = divmod(c, HGROUPS)
        bs = slice(gb * BPC, (gb + 1) * BPC)
        hs = slice(gh * HPC, (gh + 1) * HPC)
        in_maps.append(
            {
                "qT": qT[bs], "kT": kT[bs], "vT": vT[bs], "mbT": mbT[bs],
                "Wn": np.ascontiguousarray(Wn[hs]),
                "WaT": np.ascontiguousarray(WaT[hs]),
                "ones": ones,
                "bb": np.ascontiguousarray(bb[hs]),
            }
        )
    return in_maps


def _run(inputs, trace=False):
    if "nc" not in _CACHE:
        _CACHE["nc"] = _build()
    nc = _CACHE["nc"]
    in_maps = _prep_inputs(**inputs)
    last_err = None
    for _attempt in range(3):
        try:
            res = run_bass_kernel_spmd(
                nc, in_maps, core_ids=list(range(NCORES)), trace=trace
            )
            break
        except Exception as e:  # transient NRT device errors happen; retry
            last_err = e
    else:
        raise last_err
    out = np.empty((B, L, H * D), np.float32)
    for c in range(NCORES):
        gb, gh = divmod(c, HGROUPS)
        out[gb * BPC : (gb + 1) * BPC, :, gh * HPC * D : (gh + 1) * HPC * D] = (
            res.results[c]["out"]
        )
    return out, res


def kernel(**inputs) -> np.ndarray:
    out, _ = _run(inputs, trace=False)
    return out



# revision 2
# speedup vs baseline: 1.5426x; 1.5426x over previous
"""Trainium2 Bass kernel for nn_Cross_attention2 (dense transformer cross-attention).

Math (per batch b, head h), faithful to the reference module:
    Q = q @ W_h + b_h ; K = k @ W_h + b_h ; V = v @ W_h + b_h
    alpha = (Q K^T)/sqrt(512); masked -> -1e9; alpha /= sqrt(512); P = softmax(alpha)
    out[b, :, h*512:(h+1)*512] = P @ V

Device algorithm (algebraically identical post-softmax), all big matmuls in
fp8-e4m3 DoubleRow mode (2x PE throughput; fp32 PSUM accumulate):
    G    = W_h W_h^T, Wb = W_h b_h           (host, fp32; -> fp8)
    Z    = G q^T + Wb 1^T                     [512, Lq]  (q-dependent score term)
    s^T  = k Z   = (QK^T)^T minus softmax-invariant per-q shifts
    P    = exp(s^T/512 [+ mb^T/512])          (unnormalized; scores are tiny, no max-sub)
    Phat = P - 1                              (small values -> fp8 keeps the signal)
    sums = 512 + colsum(Phat)                 (tiny ones-matmuls)
    Onum = Phat^T (v W_h)  +  1 (x) T'        with T' = colsum(v) W_h  (host, exact)
    out  = Onum / sums + b_h                  (fused (psum*rsb)+bb on DVE)
The Phat split keeps fp8 quantization error ~O(|Phat|)/sums ~ 1e-4 instead of
O(1)/sqrt(512) which would fail tolerance.
Sharding: 2 batch-groups x 4 head-groups; 8 batches x 2 heads per core.
"""

import os
import sys
from contextlib import ExitStack

import numpy as np
import ml_dtypes

for _p in ("/opt/trn_rl_repo",):
    if os.path.isdir(_p) and _p not in sys.path:
        sys.path.append(_p)

import concourse.bacc as bacc
import concourse.mybir as mybir
import concourse.tile as tile
from concourse.bass import ts
from concourse.bass_utils import run_bass_kernel_spmd

dt = mybir.dt
F8 = ml_dtypes.float8_e4m3

B, L, D, H = 16, 512, 512, 8
NCORES = 8
BGROUPS, HGROUPS = 2, 4          # core grid: 2 batch-groups x 4 head-groups
BPC = B // BGROUPS               # 8 batches per core
HPC = H // HGROUPS               # 2 heads per core
C = D // 128                     # 128-row chunks per 512
NEG_MASK = -1e9 * float(np.sqrt(512.0))  # additive bias for masked entries (pre /512)

_CACHE = {}


def _build(masked: bool):
    nc = bacc.Bacc("TRN2", target_bir_lowering=False, debug=False, num_devices=NCORES)
    f32 = dt.float32
    f16 = dt.float16
    f8 = dt.float8e4
    DR = mybir.MatmulPerfMode.DoubleRow
    EXP = mybir.ActivationFunctionType.Exp
    COPY = mybir.ActivationFunctionType.Copy
    MUL = mybir.AluOpType.mult
    ADD = mybir.AluOpType.add

    qT_d = nc.dram_tensor("qT", [BPC, D, L], f8, kind="ExternalInput").ap()
    kT_d = nc.dram_tensor("kT", [BPC, D, L], f8, kind="ExternalInput").ap()
    vT_d = nc.dram_tensor("vT", [BPC, D, L], f8, kind="ExternalInput").ap()
    G_d = nc.dram_tensor("G", [HPC, D, D], f8, kind="ExternalInput").ap()
    W_d = nc.dram_tensor("W", [HPC, D, D], f8, kind="ExternalInput").ap()
    Wb_d = nc.dram_tensor("Wb", [HPC, 128, C], f32, kind="ExternalInput").ap()
    T_d = nc.dram_tensor("T", [BPC, HPC, D], f16, kind="ExternalInput").ap()
    bb_d = nc.dram_tensor("bb", [HPC, 128, D], f32, kind="ExternalInput").ap()
    ones8_d = nc.dram_tensor("ones8", [128, C, 2], f8, kind="ExternalInput").ap()
    one16_d = nc.dram_tensor("one16", [1, 128], f16, kind="ExternalInput").ap()
    if masked:
        mbT_d = nc.dram_tensor("mbT", [BPC, L, L], f32, kind="ExternalInput").ap()
    out_d = nc.dram_tensor("out", [BPC, L, HPC * D], f32, kind="ExternalOutput").ap()

    with tile.TileContext(nc) as tc, ExitStack() as ctx:
        const = ctx.enter_context(tc.tile_pool(name="const", bufs=1))
        headp = ctx.enter_context(tc.tile_pool(name="headp", bufs=1))
        acts = ctx.enter_context(tc.tile_pool(name="acts", bufs=2))
        work = ctx.enter_context(tc.tile_pool(name="work", bufs=2))
        psb = ctx.enter_context(tc.tile_pool(name="psb", bufs=4, space="PSUM"))
        pso = ctx.enter_context(tc.tile_pool(name="pso", bufs=2, space="PSUM"))
        pss = ctx.enter_context(tc.tile_pool(name="pss", bufs=1, space="PSUM"))

        # ---- weight/constant loads (PE's first MMs need G[0] + q[0] only) ----
        Gs = [headp.tile([128, C, D], f8, tag=f"G{h}", name=f"Gs{h}") for h in range(HPC)]
        nc.sync.dma_start(Gs[0][:], G_d[0].rearrange("(c p) d -> p c d", p=128))

        def load_q(b):
            tq = acts.tile([128, C, L], f8, tag="q", name=f"qTs{b}")
            nc.sync.dma_start(tq[:], qT_d[b].rearrange("(c p) q -> p c q", p=128))
            return tq

        def load_kvm(b, tq):
            tk = acts.tile([128, C, L], f8, tag="k", name=f"kTs{b}")
            nc.sync.dma_start(tk[:], kT_d[b].rearrange("(c p) q -> p c q", p=128))
            tv = acts.tile([128, C, L], f8, tag="v", name=f"vTs{b}")
            nc.sync.dma_start(tv[:], vT_d[b].rearrange("(c p) q -> p c q", p=128))
            tT = acts.tile([1, HPC, D], f16, tag="T", name=f"Ts{b}")
            nc.sync.dma_start(tT[:], T_d[b : b + 1])
            if masked:
                tm = acts.tile([128, C, L], f32, tag="m", name=f"mbs{b}")
                nc.sync.dma_start(tm[:], mbT_d[b].rearrange("(c p) q -> p c q", p=128))
            else:
                tm = None
            return (tq, tk, tv, tT, tm)

        _tq0 = load_q(0)
        nc.sync.dma_start(Gs[1][:], G_d[1].rearrange("(c p) d -> p c d", p=128))
        cur_acts = load_kvm(0, _tq0)
        Ws, Wbs, bbs = [], [], []
        for h in range(HPC):
            w = headp.tile([128, C, D], f8, tag=f"W{h}", name=f"Ws{h}")
            nc.sync.dma_start(w[:], W_d[h].rearrange("(c p) d -> p c d", p=128))
            Ws.append(w)
            wb = headp.tile([128, C], f32, tag=f"Wb{h}", name=f"Wbs{h}")
            nc.sync.dma_start(wb[:], Wb_d[h])
            Wbs.append(wb)
            bb = headp.tile([128, D], f32, tag=f"bb{h}", name=f"bbs{h}")
            nc.sync.dma_start(bb[:], bb_d[h])
            bbs.append(bb)
        ones8 = const.tile([128, C, 2], f8, tag="ones8")
        nc.sync.dma_start(ones8[:], ones8_d)
        one16 = const.tile([1, 128], f16, tag="one16")
        nc.sync.dma_start(one16[:], one16_d)

        def emit_sums_O(st, final=False):
            """sums + O for a finished (b, h); tiny sums MMs interleaved with
            the big O MMs. out = (Onum * 1/sums) + b via fused DVE op."""
            PT8, V8, tT, b, h = st

            def out_tile(u, ops, rsb, ru, split=1):
                Osb = work.tile([128, D], f32, tag="O", bufs=3, name=f"Osb{u}")
                nc.vector.scalar_tensor_tensor(
                    Osb[:], ops[:], rsb[:, 2 * ru : 2 * ru + 1], bbs[h][:], MUL, ADD
                )
                w = D // split
                for j in range(split):
                    nc.sync.dma_start(
                        out_d[b, ts(u, 128), h * D + j * w : h * D + (j + 1) * w],
                        Osb[:, j * w : (j + 1) * w],
                    )

            groups = [range(C)] if not final else [(0, 1), (2, 3)]
            for grp in groups:
                grp = list(grp)
                sums = pss.tile([128, 8], f32, tag="sums", name=f"sums{grp[0]}{final}")
                srec = work.tile([128, 8], f32, tag="srec", name=f"srec{grp[0]}{final}")
                rsb = work.tile([128, 8], f32, tag="rsb", name=f"rsb{grp[0]}{final}")
                opss = {}
                n = 0
                for u in grp:
                    opss[u] = pso.tile([128, D], f32, tag="o", name=f"ops{u}")
                    for cp in range(C // 2):
                        nc.tensor.matmul(
                            opss[u][:],
                            PT8[:, 2 * cp : 2 * cp + 2, ts(u, 128)],
                            V8[:, 2 * cp : 2 * cp + 2, :],
                            start=(cp == 0), stop=False, perf_mode=DR,
                        )
                        nc.tensor.matmul(
                            sums[:, 2 * (u - grp[0]) : 2 * (u - grp[0]) + 2],
                            PT8[:, 2 * cp : 2 * cp + 2, ts(u, 128)],
                            ones8[:, 2 * cp : 2 * cp + 2, :],
                            start=(n == 0), stop=(n == len(grp) * (C // 2) - 1),
                            perf_mode=DR, skip_group_check=True,
                        )
                        n += 1
                    # rank-1: + 1 (x) T'  (colsum(v) W term, exact from host)
                    nc.tensor.matmul(
                        opss[u][:], one16[:], tT[0:1, h, :],
                        start=False, stop=True, skip_group_check=True,
                    )
                nc.vector.tensor_scalar_add(srec[:, 0 : 2 * len(grp)], sums[:, 0 : 2 * len(grp)], 512.0)
                nc.vector.reciprocal(rsb[:, 0 : 2 * len(grp)], srec[:, 0 : 2 * len(grp)])
                for u in grp:
                    out_tile(u, opss[u], rsb, u - grp[0], split=1 if not final else 2)

        pending = None
        for b in range(BPC):
            qTb, kTb, vTb, tT, mbb = cur_acts
            if b + 1 < BPC:
                nxt_acts = load_kvm(b + 1, load_q(b + 1))
            for h in range(HPC):
                # Z = G q^T (+ Wb per-partition bias on the psum->sbuf copy)
                Zsb = work.tile([128, C, L], f8, tag="Z")
                for t in range(C):
                    zps = psb.tile([128, L], f32, tag="big")
                    for cp in range(C // 2):
                        nc.tensor.matmul(
                            zps[:], Gs[h][:, 2 * cp : 2 * cp + 2, ts(t, 128)],
                            qTb[:, 2 * cp : 2 * cp + 2, :],
                            start=(cp == 0), stop=(cp == C // 2 - 1), perf_mode=DR,
                        )
                    nc.vector.tensor_scalar_add(Zsb[:, t, :], zps[:], Wbs[h][:, t : t + 1])

                # s^T = k Z ; Phat = exp(s^T/512 [+ mb/512]) - 1
                PT8 = work.tile([128, C, L], f8, tag="PT")
                for t in range(C):
                    sps = psb.tile([128, L], f32, tag="big")
                    for cp in range(C // 2):
                        nc.tensor.matmul(
                            sps[:], kTb[:, 2 * cp : 2 * cp + 2, ts(t, 128)],
                            Zsb[:, 2 * cp : 2 * cp + 2, :],
                            start=(cp == 0), stop=(cp == C // 2 - 1), perf_mode=DR,
                        )
                    sm = work.tile([128, L], f16, tag="sm", bufs=3, name=f"sm{t}")
                    if masked:
                        smm = work.tile([128, L], f32, tag="smm", bufs=2, name=f"smm{t}")
                        nc.vector.tensor_add(smm[:], sps[:], mbb[:, t, :])
                        nc.scalar.activation(sm[:], smm[:], EXP, scale=1.0 / 512.0)
                    else:
                        nc.scalar.activation(sm[:], sps[:], EXP, scale=1.0 / 512.0)
                    nc.vector.tensor_scalar_sub(PT8[:, t, :], sm[:], 1.0)

                # V = v W  (bias + colsum handled via T'/bb)
                V8 = work.tile([128, C, D], f8, tag="V")
                for t in range(C):
                    vps = psb.tile([128, D], f32, tag="big")
                    for cp in range(C // 2):
                        nc.tensor.matmul(
                            vps[:], vTb[:, 2 * cp : 2 * cp + 2, ts(t, 128)],
                            Ws[h][:, 2 * cp : 2 * cp + 2, :],
                            start=(cp == 0), stop=(cp == C // 2 - 1), perf_mode=DR,
                        )
                    nc.scalar.activation(V8[:, t, :], vps[:], COPY)

                # software pipeline: sums/O of the previous pair lands here,
                # after this pair's PE work covered its exp/sub latency
                if pending is not None:
                    emit_sums_O(pending)
                pending = (PT8, V8, tT, b, h)
            if b + 1 < BPC:
                cur_acts = nxt_acts

        emit_sums_O(pending, final=True)

    nc.compile()
    return nc


def _prep_inputs(query, key, value, mask, Wq, bq):
    f = np.float32

    def c8(x):  # TRN e4m3 (ml_dtypes.float8_e4m3 matches; clip to max normal)
        return np.clip(np.asarray(x, f), -240.0, 240.0).astype(F8)

    qT = np.ascontiguousarray(c8(np.asarray(query, f).transpose(0, 2, 1)))
    kT = np.ascontiguousarray(c8(np.asarray(key, f).transpose(0, 2, 1)))
    vT = np.ascontiguousarray(c8(np.asarray(value, f).transpose(0, 2, 1)))
    W32 = np.asarray(Wq, f)
    b32 = np.asarray(bq, f)
    G8 = c8(np.einsum("hde,hfe->hdf", W32, W32))            # [H, D, D]
    W8 = c8(W32)
    Wb = np.einsum("hde,he->hd", W32, b32)                   # [H, D]
    WbCol = np.ascontiguousarray(Wb.reshape(H, C, 128).transpose(0, 2, 1), f)  # [H,128,C]
    vsum = np.asarray(value, f).sum(axis=1)                  # [B, D]
    Tp = np.einsum("bd,hde->bhe", vsum, W32).astype(np.float16)  # [B, H, D]
    bb = np.broadcast_to(b32[:, None, :], (H, 128, D)).copy()
    ones8 = np.ones((128, C, 2), F8)
    one16 = np.ones((1, 128), np.float16)

    m = np.asarray(mask)
    masked = not bool((m != 0).all())
    if masked:
        mb = (1.0 - (m != 0).astype(f)) * f(NEG_MASK)
        mbT = np.ascontiguousarray(mb.transpose(0, 2, 1))

    in_maps = []
    for c in range(NCORES):
        gb, gh = divmod(c, HGROUPS)
        bs = slice(gb * BPC, (gb + 1) * BPC)
        hs = slice(gh * HPC, (gh + 1) * HPC)
        im = {
            "qT": qT[bs], "kT": kT[bs], "vT": vT[bs],
            "G": np.ascontiguousarray(G8[hs]),
            "W": np.ascontiguousarray(W8[hs]),
            "Wb": np.ascontiguousarray(WbCol[hs]),
            "T": np.ascontiguousarray(Tp[bs, hs]),
            "bb": np.ascontiguousarray(bb[hs]),
            "ones8": ones8, "one16": one16,
        }
        if masked:
            im["mbT"] = mbT[bs]
        in_maps.append(im)
    return in_maps, masked


def _run(inputs, trace=False):
    in_maps, masked = _prep_inputs(**inputs)
    key = "ncm" if masked else "nc"
    if key not in _CACHE:
        _CACHE[key] = _build(masked)
    nc = _CACHE[key]
    last_err = None
    for _attempt in range(3):
        try:
            res = run_bass_kernel_spmd(
                nc, in_maps, core_ids=list(range(NCORES)), trace=trace
            )
            break
        except Exception as e:  # transient NRT device errors happen; retry
            last_err = e
    else:
        raise last_err
    out = np.empty((B, L, H * D), np.float32)
    for c in range(NCORES):
        gb, gh = divmod(c, HGROUPS)
        out[gb * BPC : (gb + 1) * BPC, :, gh * HPC * D : (gh + 1) * HPC * D] = (
            res.results[c]["out"]
        )
    return out, res


def kernel(**inputs) -> np.ndarray:
    out, _ = _run(inputs, trace=False)
    return out
